# revision 27
# baseline (speedup 1.0000x reference)
"""Cross-attention (B=8, C=128, x 64x64 tokens, y 32x32 tokens) on 8 TRN2 cores.

Strategy: data-parallel over batch B (1 batch element per NeuronCore).
Per core, channels-on-partitions layout throughout (no on-chip transposes):

  xT = x[b] viewed as [C=128, N=4096]      (natural layout of NCHW)
  yT = y[b] viewed as [C=128, M=1024]
  K2[c, m] = sum_c' (Wq^T Wk)[c,c'] yT[c',m]    W2 = Wk^T Wq folded on host
  V[m, d]  = (yT^T Wv^T)[m, d] + bv[d]          then scaled by esb[m] (bf16)
  sb[m]    = scale * yT^T (Wk^T bq);  esb = exp(sb)   (bq fold; bk cancels)
  ST[m, n] = sum_c K2[c, m] xT[c, n]            scores^T, m on partitions
  PT[m, n] = exp(scale * ST[m, n])              bias-free exp -> pair-batched
  zT[d, n] = sum_m V'[m, d] PT[m, n]            accumulated over m in PSUM
  rs[n]    = sum_m esb[m] PT[m, n]              PE ones-matmul or DVE add-tree
  out[d,n] = xT[d, n] + zT[d, n] / rs[:, n]

Exp runs as [128, 1024] activations over mt-PAIRS (two adjacent PSUM banks)
to amortize the ~352-cycle ACT pipeline fill; a few pair slots instead use a
one-op Schraudolph exp on the vector engine (tensor_scalar affine ->
uint16 bit pattern == bf16 exp approximation, ~1e-3 rel err contribution).

The max-subtraction skip is safe: scores*scale ~ N(0,1), exp stays within
~e^10 of 1.0 -- far inside fp32/bf16 range.

Matmuls: scores in float32r; PV / row-sum in bf16 (pt, V bf16: +1.5e-4 l2).
"""

import os

import numpy as np

import concourse.bass as bass  # noqa: F401  (bass types used via tile/bacc)
import concourse.mybir as mybir
import concourse.tile as tile
from concourse import bacc
from concourse.bass_utils import run_bass_kernel_spmd

B = 8
C = 128
N = 64 * 64   # 4096 x-tokens per batch element
M = 32 * 32   # 1024 y-tokens per batch element
NCHUNK = 512  # psum-bank sized n chunk
NCH = N // NCHUNK  # 8
MT = M // 128      # 8 m tiles
NPAIR = MT // 2    # 4 mt-pairs per chunk
SCALE = 1.0 / float(np.sqrt(C))
FP = mybir.dt.float32
FPR = mybir.dt.float32r
BF = mybir.dt.bfloat16
U16 = mybir.dt.uint16
EXPFN = mybir.ActivationFunctionType.Exp
MUL = mybir.AluOpType.mult
ADD = mybir.AluOpType.add

# Schraudolph exp constants (bf16 bit pattern): uint16(x*A + B)
SCH_A = SCALE * (2.0 ** 7) / float(np.log(2.0))
SCH_B = 127.0 * 2.0 ** 7 - 0.5 + 4.4

# chunks whose softmax row-sum runs on the PE (ones-matmul per mt); the rest
# use a DVE bf16 add-tree + two accumulating ones-matmuls on the summed tile
RS_PE_CHUNKS = ()
# (chunk, pair) slots whose exp runs on the DVE via Schraudolph
SCHRAUD = set()

# PE warmup: a block of dependency-free 512-col fp32r matmuls that keep
# the PE continuously busy from the end of the engine preamble (~6us)
# until the y-gated projection chain is ready (~10us). The HAM clock gate
# needs ~3.4us of CONTINUOUS PE streaming to release the 4/8 throttle
# (measured: 128-col matmul blocks do NOT flip it; 512-col ones do), so
# wide matmuls it is — 8 of them cover the window without head-of-line
# blocking the projections for long.
WARMUP_MMS = 8
TAIL_MMS = 12


def _build(biased: bool):
    nc = bacc.Bacc("TRN2", target_bir_lowering=False, debug=False, num_devices=B)

    rs_pe_chunks = tuple(range(NCH)) if biased else RS_PE_CHUNKS
    schraud = SCHRAUD

    x_d = nc.dram_tensor("x", [C, N], FPR, kind="ExternalInput").ap()
    y_d = nc.dram_tensor("y", [C, M], FPR, kind="ExternalInput").ap()
    # weights packed into one tensor: [wvT | W2 | bq2 | pad] (row pitch stays
    # 8-byte aligned; a 257-col tile trips a matmul ISA alignment check)
    wp_d = nc.dram_tensor("wpack", [C, 2 * C + 2], FPR, kind="ExternalInput").ap()
    bv_d = nc.dram_tensor("bv", [1, C], FP, kind="ExternalInput").ap()
    out_d = nc.dram_tensor("out", [C, N], FP, kind="ExternalOutput").ap()

    with tile.TileContext(nc) as tc:
        with (
            tc.tile_pool(name="const", bufs=1) as cpool,
            tc.tile_pool(name="work", bufs=2) as wpool,
            tc.tile_pool(name="ps", bufs=2, space="PSUM") as ps,
            tc.tile_pool(name="ps_zt", bufs=2, space="PSUM") as ps_zt,
            tc.tile_pool(name="ps_rs", bufs=2, space="PSUM") as ps_rs,
        ):
            xT = cpool.tile([C, N], FPR)
            yT = cpool.tile([C, M], FPR)
            wpack = cpool.tile([C, 2 * C + 2], FPR)
            wvT = wpack[:, 0:C]
            w2 = wpack[:, C : 2 * C]
            bq2 = wpack[:, 2 * C : 2 * C + 1]
            bv_row = cpool.tile([1, C], FP)
            ones_col = cpool.tile([1, C], FP)
            bv_bcast = cpool.tile([C, C], FP)
            ones_sqb = cpool.tile([C, C], BF)
            K2 = cpool.tile([C, M], FPR)
            # bf16 matmul weights need contiguous-pitch tiles (walrus LDW-opt
            # rejects wide-pitch slices): one [128,128] tile per m-tile.
            Vt = [cpool.tile([C, C], BF, name=f"Vt{mt}") for mt in range(MT)]
            esbt = [cpool.tile([C, C], BF, name=f"esbt{mt}") for mt in range(MT)]
            sbm = cpool.tile([C, MT], FP)
            esb = cpool.tile([C, MT], FP)
            warm = cpool.tile([C, NCHUNK], FPR)
            ones_f = cpool.tile([C, NCHUNK], FP)

            # input DMAs first on every queue (triggers are serial per
            # engine; firing them before memsets/other work gets the
            # transfers in flight ~2us earlier). y entirely on gpsimd so it
            # only shares HBM bandwidth with wpack+wvT early on (measured:
            # launching x alongside y starved y to ~41 GB/s and delayed the
            # whole projection chain); x eighths trail on sync/gpsimd in
            # chunk-consumption order.
            nc.gpsimd.dma_start(yT[:, : M // 2], y_d[:, : M // 2])
            nc.gpsimd.dma_start(yT[:, M // 2 :], y_d[:, M // 2 :])
            nc.sync.dma_start(wpack[:, C:], wp_d[:, C:])      # W2 | bq2
            nc.scalar.dma_start(wpack[:, :C], wp_d[:, :C])    # wvT
            nc.scalar.dma_start(bv_row[:], bv_d[:])
            for c in range(4):
                a = slice(c * (N // 4), c * (N // 4) + N // 8)
                b = slice(c * (N // 4) + N // 8, (c + 1) * (N // 4))
                nc.sync.dma_start(xT[:, a], x_d[:, a])
                nc.gpsimd.dma_start(xT[:, b], x_d[:, b])

            # memsets on the vector engine: the gpsimd/sync queues are busy
            # issuing DMA triggers (~0.7us each) and would delay the warmup
            # block behind them
            nc.vector.memset(ones_f[:], 1.0)
            nc.vector.memset(ones_col[:], 1.0)
            nc.vector.memset(ones_sqb[:], 1.0)
            nc.vector.tensor_copy(warm[:], ones_f[:])

            def filler(k, cols=NCHUNK):
                # dependency-free matmuls that plug PE idle bubbles (keeps
                # the HAM clock-gate at 8/8)
                for _ in range(k):
                    fps = ps.tile([C, NCHUNK], FP, name="fps", tag="ps")
                    nc.tensor.matmul(
                        fps[:, :cols], warm[:, :C], warm[:, :cols],
                        start=True, stop=True,
                    )

            filler(WARMUP_MMS)

            def sb_proj(mt):
                # sbm[:, mt] = yT[:, msl]^T @ bq2  (raw; SCALE applied at exp)
                msl = slice(mt * 128, (mt + 1) * 128)
                sbps = ps.tile([C, 1], FP, name="sbps", tag="ps")
                nc.tensor.matmul(
                    sbps[:], yT[:, msl].bitcast(FP), bq2.bitcast(FP),
                    start=True, stop=True,
                )
                nc.vector.tensor_copy(sbm[:, mt : mt + 1], sbps[:])

            def k2_proj(j):
                sl = slice(j * NCHUNK, (j + 1) * NCHUNK)
                k2ps = ps.tile([C, NCHUNK], FP, name="k2ps", tag="ps")
                nc.tensor.matmul(k2ps[:], w2[:], yT[:, sl], start=True, stop=True)
                nc.vector.tensor_copy(K2[:, sl], k2ps[:])

            def v_mm(mt):
                # Vt[mt] = (yT_mt^T @ Wv^T + bv) as bf16 (esb scale later)
                msl = slice(mt * 128, (mt + 1) * 128)
                vps = ps.tile([C, C], FP, name="vps", tag="ps")
                nc.tensor.matmul(vps[:], yT[:, msl], wvT[:], start=True, stop=True)
                nc.vector.tensor_add(Vt[mt][:], vps[:], bv_bcast[:])

            if biased:
                # sb chain first: it feeds esb -> Vt/esbt which gate the
                # first zt matmul. Then bv/k2/v.
                for mt in range(MT):
                    sb_proj(mt)
                nc.scalar.activation(esb[:], sbm[:], EXPFN, bias=0.0, scale=SCALE)
                bvps = ps.tile([C, C], FP, name="bvps", tag="ps")
                nc.tensor.matmul(
                    bvps[:], ones_col[:], bv_row[:], start=True, stop=True
                )
                nc.vector.tensor_copy(bv_bcast[:], bvps[:])
                for mt in range(MT):
                    nc.vector.tensor_scalar_mul(
                        esbt[mt][:], ones_sqb[:], esb[:, mt : mt + 1]
                    )
                k2_proj(0)
                for mt in range(MT):
                    v_mm(mt)
                    nc.vector.tensor_scalar_mul(
                        Vt[mt][:], Vt[mt][:], esb[:, mt : mt + 1]
                    )
                k2_proj(1)
            else:
                # all biases zero (the common case): esb==1, bv==0 -> skip
                # the whole sb/esb/bv chain; Vt is a plain bf16 copy.
                k2_proj(0)
                for mt in range(MT):
                    msl = slice(mt * 128, (mt + 1) * 128)
                    vps = ps.tile([C, C], FP, name="vps", tag="ps")
                    nc.tensor.matmul(
                        vps[:], yT[:, msl], wvT[:], start=True, stop=True
                    )
                    nc.vector.tensor_copy(Vt[mt][:], vps[:])
                    if mt % 2 == 1:
                        # the V matmuls are 128-col and invisible to the HAM
                        # activity monitor; interleaved wide fillers keep the
                        # clock gate from re-tripping mid-ramp (measured as a
                        # recurring 3.4us half-clock dip at ~16us)
                        filler(1)
                k2_proj(1)

            # attention main loop: per 512-col n-chunk j, mt-pairs p=0..3.
            # Chunk finish (rs ones-matmuls + epilogue) is DEFERRED until
            # the next chunk's first st pair is queued: the rs matmuls wait
            # on the DVE add-tree, and placing them before the next chunk's
            # st matmuls in the in-order PE queue caused a ~2us PE bubble
            # per chunk boundary (which also kept re-tripping the HAM
            # throttle during the ramp).
            opair_box = [None]
            pend_box = [None]

            def finish_chunk(pend):
                j, zt, rs, s01, rs_on_pe = pend
                nsl = slice(j * NCHUNK, (j + 1) * NCHUNK)
                if not rs_on_pe:
                    # fold the two halves of the summed pair tile (mt-even /
                    # mt-odd sums over the same n-range) into the rs bank
                    nc.tensor.matmul(
                        rs[:], ones_sqb[:], s01[:, :NCHUNK], start=True, stop=False
                    )
                    nc.tensor.matmul(
                        rs[:], ones_sqb[:], s01[:, NCHUNK:], start=False, stop=True
                    )
                # epilogue: out = x + zt/rs, written into a pair buffer so
                # stores go out as [C, 1024] transfers; the very last chunk
                # is processed in quarters so the tail-exposed stores are
                # short.
                if j % 2 == 0:
                    opair_box[0] = wpool.tile(
                        [C, 2 * NCHUNK], FP, name="opair", tag="opair", bufs=2
                    )
                opair = opair_box[0]
                half = opair[:, (j % 2) * NCHUNK : (j % 2 + 1) * NCHUNK]
                if j == NCH - 1:
                    for q in range(2):
                        qn = NCHUNK // 2
                        qs = slice(q * qn, (q + 1) * qn)
                        gq = slice(j * NCHUNK + q * qn, j * NCHUNK + (q + 1) * qn)
                        hq = half[:, qs]
                        recip = wpool.tile([C, qn], FP, name="recip", tag="recip")
                        nc.vector.reciprocal_approx_fast(recip[:], rs[:, qs])
                        nc.vector.tensor_mul(hq, zt[:, qs], recip[:])
                        nc.vector.tensor_add(hq, hq, xT[:, gq].bitcast(FP))
                        qeng = nc.sync if q == 0 else nc.scalar
                        qeng.dma_start(out_d[:, gq], hq)
                else:
                    recip = wpool.tile([C, NCHUNK], FP, name="recip", tag="recip")
                    nc.vector.reciprocal_approx_fast(recip[:], rs[:])
                    nc.vector.tensor_mul(half, zt[:], recip[:])
                    nc.vector.tensor_add(half, half, xT[:, nsl].bitcast(FP))
                    if j % 2 == 1:
                        peng = nc.sync if (j // 2) % 2 == 0 else nc.gpsimd
                        psl = slice((j - 1) * NCHUNK, (j + 1) * NCHUNK)
                        peng.dma_start(out_d[:, psl], opair[:])
                    elif j == NCH - 2:
                        nc.gpsimd.dma_start(out_d[:, nsl], half)

            # flat pipeline over all chunk*pair slots: st/exp for slot g are
            # queued BEFORE the zt matmuls of slot g-1, so the ACT engine
            # always has the next pair's scores ready and runs back-to-back
            # while the PE interleaves st/zt with one pair of lookahead.
            NG = NCH * NPAIR
            state = {}  # per-chunk live tiles
            prev = None
            for g in range(NG + 1):
                if g < NG:
                    j, p = g // NPAIR, g % NPAIR
                    nsl = slice(j * NCHUNK, (j + 1) * NCHUNK)
                    if p == 0:
                        state[j] = dict(
                            zt=ps_zt.tile([C, NCHUNK], FP, name="zt", tag="zt"),
                            rs=ps_rs.tile([C, NCHUNK], FP, name="rs", tag="rs"),
                            pts=[], s01=None,
                        )
                    stt = ps.tile([C, 2 * NCHUNK], FP, name="st", tag="ps")
                    nc.tensor.matmul(
                        stt[:, :NCHUNK], K2[:, 2 * p * 128 : (2 * p + 1) * 128],
                        xT[:, nsl], start=True, stop=True,
                    )
                    nc.tensor.matmul(
                        stt[:, NCHUNK:], K2[:, (2 * p + 1) * 128 : (2 * p + 2) * 128],
                        xT[:, nsl], start=True, stop=True,
                    )
                    if (j, p) in schraud:
                        ptu = wpool.tile(
                            [C, 2 * NCHUNK], U16, name="ptu", tag="pts", bufs=2
                        )
                        nc.vector.tensor_scalar(
                            ptu[:], stt[:], SCH_A, SCH_B, op0=MUL, op1=ADD
                        )
                        pt = ptu.bitcast(BF)
                    else:
                        pt = wpool.tile(
                            [C, 2 * NCHUNK], BF, name="pt", tag="pt", bufs=6
                        )
                        nc.scalar.activation(
                            pt[:], stt[:], EXPFN, bias=0.0, scale=SCALE
                        )
                    state[j]["pts"].append(pt)
                if prev is not None:
                    jp, pp = prev
                    sp = state[jp]
                    ma, mb = 2 * pp, 2 * pp + 1
                    ptp = sp["pts"][pp]
                    nc.tensor.matmul(
                        sp["zt"][:], Vt[ma][:], ptp[:, :NCHUNK],
                        start=(pp == 0), stop=False,
                    )
                    nc.tensor.matmul(
                        sp["zt"][:], Vt[mb][:], ptp[:, NCHUNK:],
                        start=False, stop=(pp == NPAIR - 1),
                    )
                    if jp in rs_pe_chunks:
                        nc.tensor.matmul(
                            sp["rs"][:], esbt[ma][:], ptp[:, :NCHUNK],
                            start=(pp == 0), stop=False,
                        )
                        nc.tensor.matmul(
                            sp["rs"][:], esbt[mb][:], ptp[:, NCHUNK:],
                            start=False, stop=(pp == NPAIR - 1),
                        )
                    else:
                        # incremental DVE bf16 add-tree
                        if pp == 1:
                            sp["s01"] = wpool.tile(
                                [C, 2 * NCHUNK], BF, name="s01", tag="tr1"
                            )
                            nc.vector.tensor_add(
                                sp["s01"][:], sp["pts"][0][:], sp["pts"][1][:]
                            )
                        elif pp >= 2:
                            nc.vector.tensor_add(
                                sp["s01"][:], sp["s01"][:], sp["pts"][pp][:]
                            )
                    if pp == NPAIR - 1:
                        pend_box[0] = (
                            jp, sp["zt"], sp["rs"], sp["s01"],
                            jp in rs_pe_chunks,
                        )
                        del state[jp]
                elif g == NG:
                    pass
                if g < NG and (g % NPAIR) == 1 and pend_box[0] is not None:
                    finish_chunk(pend_box[0])
                    pend_box[0] = None
                prev = (g // NPAIR, g % NPAIR) if g < NG else None
            finish_chunk(pend_box[0])

            # keep the PE (and thus the HAM clock) busy while the tail
            # epilogue + output DMAs drain
            filler(TAIL_MMS)

    nc.compile()
    return nc


_CACHE = {}


def _get_nc(biased=False):
    key = ("nc", biased)
    if key not in _CACHE:
        _CACHE[key] = _build(biased)
    return _CACHE[key]


def _make_in_maps(inputs):
    x = np.ascontiguousarray(np.asarray(inputs["x"], np.float32)).reshape(B, C, N)
    y = np.ascontiguousarray(np.asarray(inputs["y"], np.float32)).reshape(B, C, M)
    wq = np.asarray(inputs["Wq"], np.float32)
    wk = np.asarray(inputs["Wk"], np.float32)
    wvT = np.asarray(inputs["Wv"], np.float32).T
    bq = np.asarray(inputs["bq"], np.float32).reshape(C)
    bv = np.ascontiguousarray(np.asarray(inputs["bv"], np.float32).reshape(1, C))
    w2 = wk.T @ wq                      # K2 = W2^T yT
    bq2 = (wk.T @ bq).reshape(C, 1)     # sb = scale * yT^T bq2
    pad = np.zeros((C, 1), np.float32)
    wpack = np.ascontiguousarray(np.concatenate([wvT, w2, bq2, pad], axis=1))
    return [
        {
            "x": np.ascontiguousarray(x[b]),
            "y": np.ascontiguousarray(y[b]),
            "wpack": wpack,
            "bv": bv,
        }
        for b in range(B)
    ]


def _run(inputs, trace=False, **kwargs):
    biased = bool(np.any(np.asarray(inputs["bq"], np.float32)))
    nc = _get_nc(biased)
    in_maps = _make_in_maps(inputs)
    last_err = None
    for attempt in range(3):
        try:
            res = run_bass_kernel_spmd(
                nc, in_maps, list(range(B)), trace=trace, **kwargs
            )
            break
        except Exception as e:  # transient NRT device wedge: retry
            last_err = e
            if attempt == 2:
                raise
            import time

            time.sleep(15)
    out = np.stack(
        [np.asarray(res.results[b]["out"], np.float32).reshape(C, 64, 64)
         for b in range(B)]
    )
    return out, res


def kernel(**inputs) -> np.ndarray:
    out, _ = _run(inputs, trace=False)
    return out


if __name__ == "__main__":
    # smoke: build only
    os.environ.setdefault("BASS_NEVER_TRACE", "")
    _get_nc()
    print("build ok")
